# revision 14
# baseline (speedup 1.0000x reference)
"""Trainium2 Bass kernel for nn_AttentionLayer (sparse_attention).

reference:
    S[i,o]   = sum_k key[b,i,0,k] * query[b,0,o,k] / sqrt(64)
    W        = softmax(S, axis=i)                      # weights output [B, NI, NO]
    vals     = einsum("bio,biv->bov", W, value)
    vals     = vals / (||vals||_2(axis=v) + 1e-12)
    values   = swapaxes(vals, 1, 2)                    # [B, V, NO]
    returns (values, W)

Sharding: data-parallel over batch, one batch element per NeuronCore (B=8).

Per-core pipeline (bf16 matmul path, f32 accumulation):
  - warmup burst of dummy matmuls so the PE HAM un-throttles to 2.4 GHz
  - KT2/QT2: K^T/Q^T bf16 with k on partitions, duplicated into both
    64-partition halves so mm1 runs two i-tiles concurrently via PE row tiling
  - per o-chunk c (4 x 512):
      per i-tile pair (8 x 2x128):
        S (2 tile_position matmuls, K=64) -> [128, 1024] f32 PSUM
        U = exp(0.125*S) -> bf16 SBUF     (one ACT op per pair)
        vals_ps += matmul(lhsT=V'[t], rhs=U[t])  # [65, 512]; V' ones column
                                                 # -> row 64 = softmax denom
      recip = exp(-ln(colsum))  (ACT, same table set as exp)
      rep   = ones x recip      (PE K=1 broadcast, f32r)
      W     = U * rep           (DVE bf16 2x) -> DMA out, bf16->f32 SWDGE cast
      invn  = exp(-0.5*ln(ones_matmul(vals^2)))
      values[:, c] = vals * bcast(invn)  -> one DMA at end
  The softmax denominator cancels inside the L2 norm, so the values path
  uses unnormalized vals directly.
"""

import numpy as np

B, NI, NO, K, V = 8, 2048, 2048, 64, 64
P = 128          # partitions / i-tile height
T = NI // P      # 16 i-tiles
NP = T // 2      # 8 i-tile pairs
CH = 512         # o-chunk width (one PSUM bank of f32)
NCH = NO // CH   # 4 chunks
HALF = T // 2    # W scale/DMA granularity (half a chunk)
WARMUP_MM = 10

_CACHE = {}


def _build():
    import concourse.bass as bass
    import concourse.tile as tile
    from concourse import bacc, mybir
    from concourse.masks import make_identity

    f32 = mybir.dt.float32
    bf16 = mybir.dt.bfloat16
    AF = mybir.ActivationFunctionType

    nc = bacc.Bacc("TRN2", target_bir_lowering=False, debug=False, num_devices=8)
    key_d = nc.declare_dram_parameter("key", [NI, K], f32, isOutput=False)
    query_d = nc.declare_dram_parameter("query", [NO, K], f32, isOutput=False)
    value_d = nc.declare_dram_parameter("value", [NI, V], f32, isOutput=False)
    values_o = nc.declare_dram_parameter("values_out", [V, NO], f32, isOutput=True)
    weights_o = nc.declare_dram_parameter("weights_out", [NI, NO], f32, isOutput=True)

    with tile.TileContext(nc) as tc:
        with (
            tc.tile_pool(name="const", bufs=1) as constp,
            tc.tile_pool(name="stage", bufs=2) as stagep,
            tc.tile_pool(name="u", bufs=2) as up,
            tc.tile_pool(name="w", bufs=2) as wp,
            tc.tile_pool(name="small", bufs=2) as smallp,
            tc.tile_pool(name="ps_s", bufs=2, space="PSUM") as ps_s,
            tc.tile_pool(name="ps_v", bufs=2, space="PSUM") as ps_v,
            tc.tile_pool(name="ps_m", bufs=2, space="PSUM") as ps_m,
        ):
            # Pin the ACT spline table set to one containing Exp+Ln+Copy so the
            # compiler never has to switch sets mid-kernel (~2.7us per switch).
            try:
                from concourse.hw_specs import get_activation_tables

                tabs = get_activation_tables(nc.m.arch)
                want = {AF.Exp, AF.Ln, AF.Copy}
                set_id = next(
                    i for i, fns in enumerate(tabs.values()) if want <= fns
                )
                nc.scalar.add_instruction(
                    mybir.InstLoadActFuncSet(
                        name=nc.get_next_instruction_name(),
                        act_func_set_id=set_id,
                        ins=[],
                        outs=[],
                    )
                )
            except Exception:
                pass  # fall back to compiler-inserted table loads

            ident = constp.tile([P, P], bf16)
            make_identity(nc, ident[:])
            ones_row = constp.tile([1, P], bf16)
            nc.vector.memset(ones_row[:], 1.0)
            ones_col = constp.tile([K, 1], bf16)
            nc.vector.memset(ones_col[:], 1.0)

            # PE warmup: dense dummy matmuls to trip the HAM to K=8/8.
            wu_src = constp.tile([K, CH], bf16)
            nc.vector.memset(wu_src[:], 0.0)
            wu_ps = ps_m.tile([P, CH], f32, tag="misc")
            for _ in range(WARMUP_MM):
                nc.tensor.matmul(
                    wu_ps[:], wu_src[:, 0:P], wu_src[:], start=True, stop=True
                )
            # K^T/Q^T bf16, duplicated across both 64-partition halves:
            #   KT2[0:64, pt*128:+128]  = K^T tile 2*pt
            #   KT2[64:128, pt*128:+128]= K^T tile 2*pt+1
            #   QT2[0:64, :] = Q^T ; QT2[64:128, :] = copy of Q^T
            kt2 = constp.tile([P, NP * P], bf16)
            qt2 = constp.tile([P, NO], bf16)

            kstg = stagep.tile([P, T * K], bf16, tag="kstg")
            kstg3 = kstg[:].rearrange("p (g f) -> p g f", g=NP)  # f = (two k)
            qstg = stagep.tile([P, T * K], bf16, tag="qstg")
            qstg3 = qstg[:].rearrange("p (t k) -> p t k", t=T)
            qstg_f = stagep.tile([P, T * K], f32, tag="qstg_f")
            kstg_f = stagep.tile([P, T * K], f32, tag="kstg_f")
            nc.sync.dma_start(
                kstg_f[:].rearrange("p (g f) -> p g f", g=NP),
                key_d.ap().rearrange("(g p two) k -> p g (two k)", g=NP, p=P),
            )
            nc.scalar.dma_start(
                qstg_f[:].rearrange("p (t k) -> p t k", t=T),
                query_d.ap().rearrange("(t p) k -> p t k", p=P),
            )
            nc.vector.tensor_copy(kstg[:], kstg_f[:])
            nc.gpsimd.tensor_copy(qstg[:], qstg_f[:])

            # V' = [value | ones] bf16, per i-tile: [128, 65]
            vp = constp.tile([P, T * (V + 1)], bf16)
            vp3 = vp[:].rearrange("p (t c) -> p t c", t=T)
            nc.vector.memset(vp3[:, :, V : V + 1], 1.0)
            vstg = stagep.tile([P, T * V], f32, tag="vstg")
            nc.sync.dma_start(
                vstg[:].rearrange("p (g f) -> p g f", g=NP),
                value_d.ap().rearrange("(g p two) v -> p g (two v)", g=NP, p=P),
            )
            nc.vector.tensor_copy(
                vp3[:, :, 0:V],
                vstg[:].rearrange("p (t v) -> p t v", t=T),
            )


            def q_group(g):  # Q tiles 4g..4g+3 -> QT2 cols [512g, 512g+512)
                tp = ps_m.tile([K, 4 * P], bf16, tag="misc")
                for j in range(4):
                    nc.tensor.transpose(
                        tp[:, j * P : (j + 1) * P], qstg3[:, 4 * g + j, :], ident[:]
                    )
                csl = slice(g * 4 * P, (g + 1) * 4 * P)
                nc.vector.tensor_copy(qt2[0:K, csl], tp[:])
                nc.vector.tensor_copy(qt2[K:P, csl], tp[:])

            def k_pair(pt):  # paired group pt -> KT2 col block pt (both halves)
                tp = ps_m.tile([P, P], bf16, tag="misc")
                nc.tensor.transpose(tp[:], kstg3[:, pt, :], ident[:])
                nc.vector.tensor_copy(kt2[:, pt * P : (pt + 1) * P], tp[:])
                nc.tensor.matmul(
                    wu_ps[:], wu_src[:, 0:P], wu_src[:], start=True, stop=True
                )

            q_group(0)  # chunk 0 needs Q tiles 0-3 first
            for pt in range(NP):
                k_pair(pt)
            for g in range(1, 4):
                q_group(g)

            for c in range(NCH):
                for _ in range(4):
                    nc.tensor.matmul(
                        wu_ps[:], wu_src[:, 0:P], wu_src[:], start=True, stop=True
                    )
                u = up.tile([P, T * CH], bf16, tag="u")
                u3 = u[:].rearrange("p (t n) -> p t n", t=T)
                vals_ps = ps_v.tile([V + 1, CH], f32, tag="vals")
                qsl = slice(c * CH, (c + 1) * CH)
                for pt in range(NP):
                    s_ps = ps_s.tile([P, 2 * CH], f32, tag="s")
                    for j in range(2):
                        nc.tensor.matmul(
                            s_ps[:, j * CH : (j + 1) * CH],
                            kt2[j * K : (j + 1) * K, pt * P : (pt + 1) * P],
                            qt2[j * K : (j + 1) * K, qsl],
                            start=True,
                            stop=True,
                        )
                    nc.scalar.activation(
                        u3[:, 2 * pt : 2 * pt + 2, :].rearrange("p t n -> p (t n)"),
                        s_ps[:],
                        AF.Exp,
                        scale=0.125,
                    )
                    for j in range(2):
                        t = 2 * pt + j
                        nc.tensor.matmul(
                            vals_ps[:],
                            vp3[:, t, :],
                            u3[:, t, :],
                            start=(t == 0),
                            stop=(t == T - 1),
                        )

                # weights = U * (1/colsum), recip via exp(-ln) to stay in the
                # exp table set (ACT Reciprocal is banned, DVE reciprocal on a
                # single partition costs 3.4us)
                lncs = smallp.tile([1, CH], f32, tag="lncs")
                nc.scalar.activation(lncs[:], vals_ps[V : V + 1, :], AF.Ln)
                recip = smallp.tile([1, CH], bf16, tag="recip")
                nc.scalar.activation(recip[:], lncs[:], AF.Exp, scale=-1.0)
                rep_ps = ps_m.tile([P, CH], f32, tag="misc")
                nc.tensor.matmul(
                    rep_ps[:], ones_row[:, :], recip[:], start=True, stop=True
                )
                rep = smallp.tile([P, CH], bf16, tag="rep")
                nc.vector.tensor_copy(rep[:], rep_ps[:])

                w = wp.tile([P, T * CH], bf16, tag="w")
                w3 = w[:].rearrange("p (t n) -> p t n", t=T)
                wo4 = weights_o.ap().rearrange(
                    "(g p two) o -> p g two o", g=NP, p=P
                )
                w4 = w[:].rearrange("p (g two n) -> p g two n", g=NP, two=2)
                for h in range(T // HALF):
                    tsl = slice(h * HALF, (h + 1) * HALF)
                    gsl = slice(h * (NP // 2), (h + 1) * (NP // 2))
                    rep_b = rep[:].unsqueeze(1).broadcast_to([P, HALF, CH])
                    nc.vector.tensor_mul(w3[:, tsl, :], u3[:, tsl, :], rep_b)
                    for jj in range(2):
                        nc.gpsimd.dma_start(
                            wo4[:, gsl, jj, c * CH : (c + 1) * CH],
                            w4[:, gsl, jj, :],
                        )

                # values = vals * rsqrt(sum_v vals^2), denominators cancel
                vals_sb = smallp.tile([V, CH], f32, tag="vals_sb")
                nc.vector.tensor_copy(vals_sb[:], vals_ps[0:V, :])
                sq = smallp.tile([V, CH], bf16, tag="sq")
                nc.vector.tensor_mul(sq[:], vals_sb[:], vals_sb[:])
                ss_ps = ps_m.tile([1, CH], f32, tag="misc")
                nc.tensor.matmul(
                    ss_ps[:], ones_col[:, :], sq[:], start=True, stop=True
                )
                lnt = smallp.tile([1, CH], f32, tag="lnt")
                nc.scalar.activation(lnt[:], ss_ps[:], AF.Ln)
                invn = smallp.tile([1, CH], bf16, tag="invn")
                nc.scalar.activation(invn[:], lnt[:], AF.Exp, scale=-0.5)
                repn_ps = ps_m.tile([V, CH], f32, tag="misc")
                nc.tensor.matmul(
                    repn_ps[:], ones_row[:, 0:V], invn[:], start=True, stop=True
                )
                vouts = smallp.tile([V, CH], f32, tag="vouts")
                nc.vector.tensor_mul(vouts[:], vals_sb[:], repn_ps[:])
                nc.sync.dma_start(values_o.ap()[:, c * CH : (c + 1) * CH], vouts[:])

            # keep the verifier happy: PSUM locations need a reader
            nc.vector.tensor_copy(wu_src[0:1, 0:1], wu_ps[0:1, 0:1])

    nc.compile()
    return nc


def get_nc():
    if "nc" not in _CACHE:
        _CACHE["nc"] = _build()
    return _CACHE["nc"]


def kernel(key, query, value):
    from concourse.bass_utils import run_bass_kernel_spmd

    key = np.ascontiguousarray(np.asarray(key, dtype=np.float32))
    query = np.ascontiguousarray(np.asarray(query, dtype=np.float32))
    value = np.ascontiguousarray(np.asarray(value, dtype=np.float32))

    nc = get_nc()
    in_maps = [
        {
            "key": np.ascontiguousarray(key[i, :, 0, :]),
            "query": np.ascontiguousarray(query[i, 0]),
            "value": np.ascontiguousarray(value[i]),
        }
        for i in range(B)
    ]
    res = run_bass_kernel_spmd(nc, in_maps, core_ids=list(range(B)))
    values = np.stack([res.results[i]["values_out"] for i in range(B)])
    weights = np.stack([res.results[i]["weights_out"] for i in range(B)])
    return values, weights


# revision 15
# speedup vs baseline: 1.0834x; 1.0834x over previous
"""Trainium2 Bass kernel for nn_AttentionLayer (sparse_attention).

reference:
    S[i,o]   = sum_k key[b,i,0,k] * query[b,0,o,k] / sqrt(64)
    W        = softmax(S, axis=i)                      # weights output [B, NI, NO]
    vals     = einsum("bio,biv->bov", W, value)
    vals     = vals / (||vals||_2(axis=v) + 1e-12)
    values   = swapaxes(vals, 1, 2)                    # [B, V, NO]
    returns (values, W)

Sharding: data-parallel over batch, one batch element per NeuronCore (B=8).

Per-core pipeline (bf16 matmul path, f32 accumulation):
  - warmup burst of dummy matmuls so the PE HAM un-throttles to 2.4 GHz
  - KT2/QT2: K^T/Q^T bf16 with k on partitions, duplicated into both
    64-partition halves so mm1 runs two i-tiles concurrently via PE row tiling
  - per o-chunk c (4 x 512):
      per i-tile pair (8 x 2x128):
        S (2 tile_position matmuls, K=64) -> [128, 1024] f32 PSUM
        U = exp(0.125*S) -> bf16 SBUF     (one ACT op per pair)
        vals_ps += matmul(lhsT=V'[t], rhs=U[t])  # [65, 512]; V' ones column
                                                 # -> row 64 = softmax denom
      recip = exp(-ln(colsum))  (ACT, same table set as exp)
      rep   = ones x recip      (PE K=1 broadcast, f32r)
      W     = U * rep           (DVE bf16 2x) -> DMA out, bf16->f32 SWDGE cast
      invn  = exp(-0.5*ln(ones_matmul(vals^2)))
      values[:, c] = vals * bcast(invn)  -> one DMA at end
  The softmax denominator cancels inside the L2 norm, so the values path
  uses unnormalized vals directly.
"""

import numpy as np

B, NI, NO, K, V = 8, 2048, 2048, 64, 64
P = 128          # partitions / i-tile height
T = NI // P      # 16 i-tiles
NP = T // 2      # 8 i-tile pairs
CH = 512         # o-chunk width (one PSUM bank of f32)
NCH = NO // CH   # 4 chunks
HALF = T // 2    # W scale/DMA granularity (half a chunk)
WARMUP_MM = 10

_CACHE = {}


def _build():
    import concourse.bass as bass
    import concourse.tile as tile
    from concourse import bacc, mybir
    from concourse.masks import make_identity

    f32 = mybir.dt.float32
    bf16 = mybir.dt.bfloat16
    AF = mybir.ActivationFunctionType

    nc = bacc.Bacc("TRN2", target_bir_lowering=False, debug=False, num_devices=8)
    key_d = nc.declare_dram_parameter("key", [NI, K], f32, isOutput=False)
    query_d = nc.declare_dram_parameter("query", [NO, K], f32, isOutput=False)
    value_d = nc.declare_dram_parameter("value", [NI, V], f32, isOutput=False)
    values_o = nc.declare_dram_parameter("values_out", [V, NO], f32, isOutput=True)
    weights_o = nc.declare_dram_parameter("weights_out", [NI, NO], f32, isOutput=True)

    with tile.TileContext(nc) as tc:
        with (
            tc.tile_pool(name="const", bufs=1) as constp,
            tc.tile_pool(name="stage", bufs=2) as stagep,
            tc.tile_pool(name="u", bufs=3) as up,
            tc.tile_pool(name="w", bufs=2) as wp,
            tc.tile_pool(name="small", bufs=2) as smallp,
            tc.tile_pool(name="ps_s", bufs=2, space="PSUM") as ps_s,
            tc.tile_pool(name="ps_v", bufs=2, space="PSUM") as ps_v,
            tc.tile_pool(name="ps_m", bufs=2, space="PSUM") as ps_m,
        ):
            # Pin the ACT spline table set to one containing Exp+Ln+Copy so the
            # compiler never has to switch sets mid-kernel (~2.7us per switch).
            try:
                from concourse.hw_specs import get_activation_tables

                tabs = get_activation_tables(nc.m.arch)
                want = {AF.Exp, AF.Ln, AF.Copy}
                set_id = next(
                    i for i, fns in enumerate(tabs.values()) if want <= fns
                )
                nc.scalar.add_instruction(
                    mybir.InstLoadActFuncSet(
                        name=nc.get_next_instruction_name(),
                        act_func_set_id=set_id,
                        ins=[],
                        outs=[],
                    )
                )
            except Exception:
                pass  # fall back to compiler-inserted table loads

            ident = constp.tile([P, P], bf16)
            make_identity(nc, ident[:])
            ones_row = constp.tile([1, P], bf16)
            nc.vector.memset(ones_row[:], 1.0)
            ones_col = constp.tile([K, 1], bf16)
            nc.vector.memset(ones_col[:], 1.0)

            # PE warmup: dense dummy matmuls to trip the HAM to K=8/8.
            wu_src = constp.tile([K, CH], bf16)
            nc.vector.memset(wu_src[:], 0.0)
            wu_ps = ps_m.tile([P, CH], f32, tag="misc")
            for _ in range(WARMUP_MM):
                nc.tensor.matmul(
                    wu_ps[:], wu_src[:, 0:P], wu_src[:], start=True, stop=True
                )
            # K^T/Q^T bf16, duplicated across both 64-partition halves:
            #   KT2[0:64, pt*128:+128]  = K^T tile 2*pt
            #   KT2[64:128, pt*128:+128]= K^T tile 2*pt+1
            #   QT2[0:64, :] = Q^T ; QT2[64:128, :] = copy of Q^T
            kt2 = constp.tile([P, NP * P], bf16)
            qt2 = constp.tile([P, NO], bf16)

            kstg = stagep.tile([P, T * K], bf16, tag="kstg")
            kstg3 = kstg[:].rearrange("p (g f) -> p g f", g=NP)  # f = (two k)
            qstg = stagep.tile([P, T * K], bf16, tag="qstg")
            qstg3 = qstg[:].rearrange("p (t k) -> p t k", t=T)
            qstg_f = stagep.tile([P, T * K], f32, tag="qstg_f")
            kstg_f = stagep.tile([P, T * K], f32, tag="kstg_f")
            nc.sync.dma_start(
                kstg_f[:].rearrange("p (g f) -> p g f", g=NP),
                key_d.ap().rearrange("(g p two) k -> p g (two k)", g=NP, p=P),
            )
            nc.scalar.dma_start(
                qstg_f[:].rearrange("p (t k) -> p t k", t=T),
                query_d.ap().rearrange("(t p) k -> p t k", p=P),
            )
            nc.vector.tensor_copy(kstg[:], kstg_f[:])
            nc.vector.tensor_copy(qstg[:], qstg_f[:])

            # V' = [value | ones] bf16, per i-tile: [128, 65]
            vp = constp.tile([P, T * (V + 1)], bf16)
            vp3 = vp[:].rearrange("p (t c) -> p t c", t=T)
            nc.vector.memset(vp3[:, :, V : V + 1], 1.0)
            vstg = stagep.tile([P, T * V], f32, tag="vstg")
            nc.sync.dma_start(
                vstg[:].rearrange("p (g f) -> p g f", g=NP),
                value_d.ap().rearrange("(g p two) v -> p g (two v)", g=NP, p=P),
            )
            nc.vector.tensor_copy(
                vp3[:, :, 0:V],
                vstg[:].rearrange("p (t v) -> p t v", t=T),
            )


            def q_group(g):  # Q tiles 4g..4g+3 -> QT2 cols [512g, 512g+512)
                tp = ps_m.tile([K, 4 * P], bf16, tag="misc")
                for j in range(4):
                    nc.tensor.transpose(
                        tp[:, j * P : (j + 1) * P], qstg3[:, 4 * g + j, :], ident[:]
                    )
                csl = slice(g * 4 * P, (g + 1) * 4 * P)
                nc.vector.tensor_copy(qt2[0:K, csl], tp[:])
                nc.vector.tensor_copy(qt2[K:P, csl], tp[:])

            def k_pair(pt):  # paired group pt -> KT2 col block pt (both halves)
                tp = ps_m.tile([P, P], bf16, tag="misc")
                nc.tensor.transpose(tp[:], kstg3[:, pt, :], ident[:])
                nc.vector.tensor_copy(kt2[:, pt * P : (pt + 1) * P], tp[:])
                nc.tensor.matmul(
                    wu_ps[:], wu_src[:, 0:P], wu_src[:], start=True, stop=True
                )

            q_group(0)  # chunk 0 needs Q tiles 0-3 first
            for pt in range(NP):
                k_pair(pt)
            for g in range(1, 4):
                q_group(g)

            for c in range(NCH):
                u = up.tile([P, T * CH], bf16, tag="u")
                u3 = u[:].rearrange("p (t n) -> p t n", t=T)
                vals_ps = ps_v.tile([V + 1, CH], f32, tag="vals")
                qsl = slice(c * CH, (c + 1) * CH)
                for pt in range(NP):
                    s_ps = ps_s.tile([P, 2 * CH], f32, tag="s")
                    for j in range(2):
                        nc.tensor.matmul(
                            s_ps[:, j * CH : (j + 1) * CH],
                            kt2[j * K : (j + 1) * K, pt * P : (pt + 1) * P],
                            qt2[j * K : (j + 1) * K, qsl],
                            start=True,
                            stop=True,
                        )
                    nc.scalar.activation(
                        u3[:, 2 * pt : 2 * pt + 2, :].rearrange("p t n -> p (t n)"),
                        s_ps[:],
                        AF.Exp,
                        scale=0.125,
                    )
                    for j in range(2):
                        t = 2 * pt + j
                        nc.tensor.matmul(
                            vals_ps[:],
                            vp3[:, t, :],
                            u3[:, t, :],
                            start=(t == 0),
                            stop=(t == T - 1),
                        )

                # weights = U * (1/colsum), recip via exp(-ln) to stay in the
                # exp table set (ACT Reciprocal is banned, DVE reciprocal on a
                # single partition costs 3.4us)
                lncs = smallp.tile([1, CH], f32, tag="lncs")
                nc.scalar.activation(lncs[:], vals_ps[V : V + 1, :], AF.Ln)
                recip = smallp.tile([1, CH], bf16, tag="recip")
                nc.scalar.activation(recip[:], lncs[:], AF.Exp, scale=-1.0)
                rep_ps = ps_m.tile([P, CH], f32, tag="misc")
                nc.tensor.matmul(
                    rep_ps[:], ones_row[:, :], recip[:], start=True, stop=True
                )
                rep = smallp.tile([P, CH], bf16, tag="rep")
                nc.vector.tensor_copy(rep[:], rep_ps[:])

                w = wp.tile([P, T * CH], bf16, tag="w")
                w3 = w[:].rearrange("p (t n) -> p t n", t=T)
                wo4 = weights_o.ap().rearrange(
                    "(g p two) o -> p g two o", g=NP, p=P
                )
                w4 = w[:].rearrange("p (g two n) -> p g two n", g=NP, two=2)
                for h in range(T // HALF):
                    tsl = slice(h * HALF, (h + 1) * HALF)
                    gsl = slice(h * (NP // 2), (h + 1) * (NP // 2))
                    rep_b = rep[:].unsqueeze(1).broadcast_to([P, HALF, CH])
                    nc.vector.tensor_mul(w3[:, tsl, :], u3[:, tsl, :], rep_b)
                    for jj in range(2):
                        nc.gpsimd.dma_start(
                            wo4[:, gsl, jj, c * CH : (c + 1) * CH],
                            w4[:, gsl, jj, :],
                        )

                # values = vals * rsqrt(sum_v vals^2), denominators cancel
                vals_sb = smallp.tile([V, CH], f32, tag="vals_sb")
                nc.vector.tensor_copy(vals_sb[:], vals_ps[0:V, :])
                sq = smallp.tile([V, CH], bf16, tag="sq")
                nc.vector.tensor_mul(sq[:], vals_sb[:], vals_sb[:])
                ss_ps = ps_m.tile([1, CH], f32, tag="misc")
                nc.tensor.matmul(
                    ss_ps[:], ones_col[:, :], sq[:], start=True, stop=True
                )
                lnt = smallp.tile([1, CH], f32, tag="lnt")
                nc.scalar.activation(lnt[:], ss_ps[:], AF.Ln)
                invn = smallp.tile([1, CH], bf16, tag="invn")
                nc.scalar.activation(invn[:], lnt[:], AF.Exp, scale=-0.5)
                repn_ps = ps_m.tile([V, CH], f32, tag="misc")
                nc.tensor.matmul(
                    repn_ps[:], ones_row[:, 0:V], invn[:], start=True, stop=True
                )
                vouts = smallp.tile([V, CH], f32, tag="vouts")
                nc.vector.tensor_mul(vouts[:], vals_sb[:], repn_ps[:])
                nc.sync.dma_start(values_o.ap()[:, c * CH : (c + 1) * CH], vouts[:])
                for _ in range(3):
                    nc.tensor.matmul(
                        wu_ps[:], wu_src[:, 0:P], wu_src[:], start=True, stop=True
                    )

            # keep the verifier happy: PSUM locations need a reader
            nc.vector.tensor_copy(wu_src[0:1, 0:1], wu_ps[0:1, 0:1])

    nc.compile()
    return nc


def get_nc():
    if "nc" not in _CACHE:
        _CACHE["nc"] = _build()
    return _CACHE["nc"]


def kernel(key, query, value):
    from concourse.bass_utils import run_bass_kernel_spmd

    key = np.ascontiguousarray(np.asarray(key, dtype=np.float32))
    query = np.ascontiguousarray(np.asarray(query, dtype=np.float32))
    value = np.ascontiguousarray(np.asarray(value, dtype=np.float32))

    nc = get_nc()
    in_maps = [
        {
            "key": np.ascontiguousarray(key[i, :, 0, :]),
            "query": np.ascontiguousarray(query[i, 0]),
            "value": np.ascontiguousarray(value[i]),
        }
        for i in range(B)
    ]
    res = run_bass_kernel_spmd(nc, in_maps, core_ids=list(range(B)))
    values = np.stack([res.results[i]["values_out"] for i in range(B)])
    weights = np.stack([res.results[i]["weights_out"] for i in range(B)])
    return values, weights
